# revision 22
# baseline (speedup 1.0000x reference)
"""AIG triple embedding layer on 8 TRN2 NeuronCores.

Math: out[t] = W @ concat(src[t], r[t], dst[t]) + b
            = W1 @ table[fs[t]] + (W2 @ edge[rel[t]] + b) + W3 @ table[fd[t]]
where table = [inp_enc(256) | out_enc(256) | gate[:256] | zeros], and
fs = src_type*256 + src_idx (type==3 rows land in the zero pad).

Because indices are bounded (idx < 256, 4 types, rel in {0,1}) the linear
layer is folded into small fused tables computed on device:
  TA[r] = table[r] @ W1.T,  TB[r] = table[r] @ W3.T   (768 nonzero rows each)
  EW2'[v] = edge[v] @ W2.T + b                        (2 rows)
then out[t] = TA[fs[t]] + TB[fd[t]] + EW2'[rel[t]] -- pure row selection.

Two device implementations (IMPL flag):
- "onehot" (default, ~0.60 ms): row selection as TensorEngine matmuls
  against {0,1} one-hot matrices built on the VectorEngine (is_equal vs
  iota), accumulating all three terms in one PSUM tile, PE-transposing
  [d,t] -> [t,d], and storing. No per-row DMA descriptors at all - the
  SWDGE Q7 descriptor generator (~8.5 ns/row) is the bottleneck of any
  gather-based variant (measured ~1.29 ms).
- "gather": dma_gather row-fetch variant, kept as a reference fallback.

Sharding: data-parallel over T across 8 cores; tables/weights replicated.
"""

import numpy as np

D = 128
T = 524288
NCORES = 8
NSHARD = T // NCORES  # 65536
NI = 256              # num_input_nodes == num_output_nodes == IDX_MAX
ROWS = 4 * NI         # 1024 padded flat-table rows (type*256 + idx < 1024)
P = 128
BLK = 8192            # triples per pipeline block
NBLK = NSHARD // BLK  # 8
JB = BLK // P         # 64 rows per partition per block
FB = BLK // 16        # 512 idx columns per block in the [16, *] wrapped layout
G = 1024              # indices per dma_gather call (SWDGE ring holds <2048)
NG = BLK // G         # 8 gather sub-calls per block per table
NF = NSHARD // 16     # 4096 idx columns whole-shard

TABLE_DT = "float16"  # float32|bfloat16|float16
IMPL = "onehot"       # "gather" | "onehot"

_CACHE = {}


def _sinusoid(n, d):
    pos = np.arange(n, dtype=np.float32)[:, None]
    div = np.exp(np.arange(0, d, 2, dtype=np.float32)
                 * (-np.log(np.float32(10000.0)) / np.float32(d)))
    ang = (pos * div).astype(np.float32)
    enc = np.zeros((n, d), np.float32)
    enc[:, 0::2] = np.sin(ang)
    enc[:, 1::2] = np.cos(ang)
    return enc


def _build_nc():
    import concourse.bacc as bacc
    import concourse.mybir as mybir
    import concourse.tile as tile

    f32 = mybir.dt.float32
    tdt = getattr(mybir.dt, TABLE_DT)
    i32 = mybir.dt.int32
    i16 = mybir.dt.int16
    AL = mybir.AluOpType

    nc = bacc.Bacc(None, target_bir_lowering=False)

    tblT = nc.dram_tensor("tblT", [P, ROWS], f32, kind="ExternalInput")
    wt = nc.dram_tensor("wt", [3 * D, D], f32, kind="ExternalInput")
    edgt = nc.dram_tensor("edget", [P, 2], f32, kind="ExternalInput")
    bv = nc.dram_tensor("bvec", [1, D], f32, kind="ExternalInput")
    s_i = nc.dram_tensor("src_idx", [NSHARD], i32, kind="ExternalInput")
    s_t = nc.dram_tensor("src_type", [NSHARD], i32, kind="ExternalInput")
    r_l = nc.dram_tensor("rel", [NSHARD], i32, kind="ExternalInput")
    d_i = nc.dram_tensor("dst_idx", [NSHARD], i32, kind="ExternalInput")
    d_t = nc.dram_tensor("dst_type", [NSHARD], i32, kind="ExternalInput")
    out = nc.dram_tensor("out", [NSHARD, D], f32, kind="ExternalOutput")

    # Triple t sits at gathered position (block b, call k, q) with
    # p = q%128 = 16*w2 + pi, j = k*8 + q//128 = k*8 + u2, and the output
    # write keeps partition p's 64 rows contiguous: t = p*512 + b*64 + j.
    # With m = b*64 + k*8 + u2 in [0,512):  t = w2*8192 + pi*512 + m.
    # Whole-shard idx arrays load once into [16, 4096] SBUF as f' = w2*512+m
    # (both sides clean 3-dim APs, 2KB contiguous DRAM runs); the int16 cast
    # later permutes to the wrapped per-call order f = m*8 + w2.
    def idx_view(h):
        return h[:].rearrange("(w2 pi m) -> pi w2 m", w2=8, pi=16, m=512)

    outv = out[:].rearrange("(p b j) d -> b p (j d)", p=P, b=NBLK, j=JB)

    with tile.TileContext(nc) as tc:
        with (
            tc.tile_pool(name="const", bufs=1) as cpool,
            tc.tile_pool(name="psum", bufs=2, space="PSUM") as psum,
            tc.tile_pool(name="setup", bufs=2) as spool,
            tc.tile_pool(name="idxin", bufs=2) as iip,
            tc.tile_pool(name="idxmath", bufs=2) as imp,
            tc.tile_pool(name="idxrep", bufs=2) as irp,
            tc.tile_pool(name="gather", bufs=2) as gpool,
            tc.tile_pool(name="sum", bufs=2) as opool,
            tc.tile_pool(name="dram", bufs=1, space="DRAM") as dpool,
        ):
            # ---------------- fused tables (one-time, tiny) ----------------
            TA = dpool.tile([2 * ROWS, D], tdt)
            TB = dpool.tile([ROWS, D], tdt)

            tblT_sb = cpool.tile([P, ROWS], f32)
            nc.sync.dma_start(out=tblT_sb[:], in_=tblT[:])
            wt_sb = cpool.tile([P, 3 * D], f32)  # three [128,128] chunks
            for k in range(3):
                nc.sync.dma_start(out=wt_sb[:, k * D:(k + 1) * D],
                                  in_=wt[k * D:(k + 1) * D, :])
            edgt_sb = cpool.tile([P, 2], f32)
            nc.sync.dma_start(out=edgt_sb[:], in_=edgt[:])
            b_sb = cpool.tile([1, D], f32)
            nc.sync.dma_start(out=b_sb[:], in_=bv[:])
            ones2 = cpool.tile([1, 2], f32)
            nc.vector.memset(ones2[:], 1.0)
            onesM = cpool.tile([1, P], f32)
            nc.vector.memset(onesM[:], 1.0)

            # EW2'[v] = edge[v] @ W2.T + b   (one [1,D] row per v, so each
            # lands at base partition 0 as required for matmul rhs use)
            ew = []
            for v in range(2):
                pe = psum.tile([1, D], f32, tag="pe")
                nc.tensor.matmul(out=pe[:], lhsT=edgt_sb[:, v:v + 1],
                                 rhs=wt_sb[:, D:2 * D], start=True, stop=False)
                nc.tensor.matmul(out=pe[:], lhsT=ones2[:, 0:1], rhs=b_sb[:],
                                 start=False, stop=True)
                ewv = cpool.tile([1, D], f32, tag=f"ew{v}")
                nc.vector.tensor_copy(out=ewv[:], in_=pe[:])
                ew.append(ewv)

            for c in range(ROWS // P):  # 8 chunks of 128 table rows
                lhs = tblT_sb[:, c * P:(c + 1) * P]
                for v in range(2):  # TA halves: + EW2'[v]
                    pa = psum.tile([P, D], f32, tag="pa")
                    nc.tensor.matmul(out=pa[:], lhsT=lhs,
                                     rhs=wt_sb[:, 0:D], start=True, stop=False)
                    nc.tensor.matmul(out=pa[:], lhsT=onesM[:],
                                     rhs=ew[v][:], start=False, stop=True)
                    av = spool.tile([P, D], tdt, tag="av")
                    nc.vector.tensor_copy(out=av[:], in_=pa[:])
                    nc.sync.dma_start(
                        out=TA[v * ROWS + c * P: v * ROWS + (c + 1) * P, :],
                        in_=av[:])
                pb = psum.tile([P, D], f32, tag="pa")
                nc.tensor.matmul(out=pb[:], lhsT=lhs,
                                 rhs=wt_sb[:, 2 * D:3 * D], start=True, stop=True)
                bt = spool.tile([P, D], tdt, tag="av")
                nc.vector.tensor_copy(out=bt[:], in_=pb[:])
                nc.sync.dma_start(out=TB[c * P:(c + 1) * P, :], in_=bt[:])

            # ---------------- per-block pipeline ----------------
            for bb in range(NBLK):
                sti = iip.tile([16, FB], i32, tag="sti")
                sii = iip.tile([16, FB], i32, tag="sii")
                rli = iip.tile([16, FB], i32, tag="rli")
                dti = iip.tile([16, FB], i32, tag="dti")
                dii = iip.tile([16, FB], i32, tag="dii")

                def split(t):  # [16, FB] -> [16, 8, 64] contiguous view
                    return t[:].rearrange("pi (w2 mw) -> pi w2 mw", w2=8, mw=JB)

                for tl, h in ((sti, s_t), (sii, s_i), (rli, r_l),
                              (dti, d_t), (dii, d_i)):
                    nc.sync.dma_start(
                        out=split(tl),
                        in_=idx_view(h)[:, :, bb * JB:(bb + 1) * JB])

                # fs = st*256 + si + rel*1024 ; fd = dt*256 + di
                fs32 = imp.tile([16, FB], i32, tag="fs32")
                t32 = imp.tile([16, FB], i32, tag="t32")
                fd32 = imp.tile([16, FB], i32, tag="fd32")
                nc.vector.tensor_scalar(out=fs32[:], in0=sti[:], scalar1=8,
                                        scalar2=None, op0=AL.logical_shift_left)
                nc.vector.tensor_tensor(out=fs32[:], in0=fs32[:], in1=sii[:],
                                        op=AL.add)
                nc.vector.tensor_scalar(out=t32[:], in0=rli[:], scalar1=10,
                                        scalar2=None, op0=AL.logical_shift_left)
                nc.vector.tensor_tensor(out=fs32[:], in0=fs32[:], in1=t32[:],
                                        op=AL.add)
                nc.vector.tensor_scalar(out=fd32[:], in0=dti[:], scalar1=8,
                                        scalar2=None, op0=AL.logical_shift_left)
                nc.vector.tensor_tensor(out=fd32[:], in0=fd32[:], in1=dii[:],
                                        op=AL.add)

                # cast to int16, permuting storage f''=w2*64+mw into the
                # wrapped per-call order f = mw*8 + w2
                def pmw(t):  # [16, 64, 8] permuted view of f'' = w2*64 + mw
                    return t[:].rearrange("pi (w2 mw) -> pi mw w2",
                                          w2=8, mw=JB)

                fsd16 = imp.tile([16, 2 * FB], i16, tag="fsd16")
                nc.vector.tensor_copy(
                    out=fsd16[:, 0:FB].rearrange("pi (mw w2) -> pi mw w2",
                                                 mw=JB, w2=8),
                    in_=pmw(fs32))
                nc.vector.tensor_copy(
                    out=fsd16[:, FB:2 * FB].rearrange("pi (mw w2) -> pi mw w2",
                                                      mw=JB, w2=8),
                    in_=pmw(fd32))

                # replicate across the 8 Q7-core partition groups
                rep = irp.tile([P, 2 * FB], i16, tag="rep")
                for g in range(8):
                    nc.sync.dma_start(out=rep[g * 16:(g + 1) * 16, :],
                                      in_=fsd16[:])

                ga = gpool.tile([P, JB, D], tdt, tag="ga")
                gb = gpool.tile([P, JB, D], tdt, tag="gb")
                for k in range(NG):
                    nc.gpsimd.dma_gather(
                        ga[:, k * (G // P):(k + 1) * (G // P), :], TA[:],
                        rep[:, k * (G // 16):(k + 1) * (G // 16)], G, G, D)
                    nc.gpsimd.dma_gather(
                        gb[:, k * (G // P):(k + 1) * (G // P), :], TB[:],
                        rep[:, FB + k * (G // 16):FB + (k + 1) * (G // 16)],
                        G, G, D)

                s = opool.tile([P, JB * D], f32, tag="s")
                nc.vector.tensor_tensor(
                    out=s[:],
                    in0=ga[:].rearrange("p a b -> p (a b)"),
                    in1=gb[:].rearrange("p a b -> p (a b)"),
                    op=AL.add)
                nc.sync.dma_start(out=outv[bb], in_=s[:])

    nc.compile()
    return nc


def _build_nc_onehot():
    """PE-selection variant: no row gathers at all.

    For each 512-triple group, build {0,1} one-hot matrices of the flat
    indices (DVE is_equal against iota columns) and contract them with the
    fused tables on the TensorEngine, accumulating src-table, dst-table and
    rel-edge contributions in one PSUM tile:
        psum[d, t] = sum_c TA_c[r, d]^T @ onehotA_c[r, t]
                   + sum_c TB_c[r, d]^T @ onehotB_c[r, t]
                   + EW2'[v, d]^T     @ onehotRel[v, t]
    then PE-transpose back to [t, d] and store. Broadcast of index values
    across partitions also rides the PE (transpose of a free-dim broadcast).
    Zero table rows (type 3 / padding) are skipped: their one-hot columns
    match no chunk, so the contribution is zero by construction.
    """
    import concourse.bacc as bacc
    import concourse.mybir as mybir
    import concourse.tile as tile
    from concourse.masks import make_identity

    f32 = mybir.dt.float32
    fp16 = mybir.dt.float16
    i32 = mybir.dt.int32
    AL = mybir.AluOpType

    NR = 768              # nonzero table rows (6 chunks of 128)
    NC_ = NR // P         # 6
    U = NSHARD // P       # 512 triples per partition-row; t = p*512 + u
    SG = 32               # super-groups of 16 u-columns (2048 triples)
    UQ = 4                # u-columns per psum tile (512 triples)

    nc = bacc.Bacc(None, target_bir_lowering=False)

    tblT = nc.dram_tensor("tblT", [P, ROWS], f32, kind="ExternalInput")
    wt = nc.dram_tensor("wt", [3 * D, D], f32, kind="ExternalInput")
    edgt = nc.dram_tensor("edget", [P, 2], f32, kind="ExternalInput")
    bv = nc.dram_tensor("bvec", [1, D], f32, kind="ExternalInput")
    ioc = nc.dram_tensor("ioc", [P, 8], f32, kind="ExternalInput")
    s_i = nc.dram_tensor("src_idx", [NSHARD], i32, kind="ExternalInput")
    s_t = nc.dram_tensor("src_type", [NSHARD], i32, kind="ExternalInput")
    r_l = nc.dram_tensor("rel", [NSHARD], i32, kind="ExternalInput")
    d_i = nc.dram_tensor("dst_idx", [NSHARD], i32, kind="ExternalInput")
    d_t = nc.dram_tensor("dst_type", [NSHARD], i32, kind="ExternalInput")
    out = nc.dram_tensor("out", [NSHARD, D], f32, kind="ExternalOutput")

    ov = out[:].rearrange("(m u) d -> m u d", m=P, u=U)

    with tile.TileContext(nc) as tc:
        with (
            tc.tile_pool(name="const", bufs=1) as cpool,
            tc.tile_pool(name="psumB", bufs=2, space="PSUM") as pB,
            tc.tile_pool(name="psumO", bufs=4, space="PSUM") as pO,
            tc.tile_pool(name="psumT", bufs=2, space="PSUM") as pT,
            tc.tile_pool(name="setup", bufs=2) as spool,
            tc.tile_pool(name="bcast", bufs=3) as xpool,
            tc.tile_pool(name="oh", bufs=4) as ohp,
            tc.tile_pool(name="ohr", bufs=5) as ohrp,
            tc.tile_pool(name="outs", bufs=4) as osp,
        ):
            # ---------------- constants + fused tables ----------------
            tblT_sb = cpool.tile([P, NR], f32)
            nc.sync.dma_start(out=tblT_sb[:], in_=tblT[:, 0:NR])
            wt_sb = cpool.tile([P, 3 * D], f32)
            for k in range(3):
                nc.sync.dma_start(out=wt_sb[:, k * D:(k + 1) * D],
                                  in_=wt[k * D:(k + 1) * D, :])
            edgt_sb = cpool.tile([P, 2], f32)
            nc.sync.dma_start(out=edgt_sb[:], in_=edgt[:])
            b_sb = cpool.tile([1, D], f32)
            nc.sync.dma_start(out=b_sb[:], in_=bv[:])
            ioc_sb = cpool.tile([P, 8], f32)
            nc.sync.dma_start(out=ioc_sb[:], in_=ioc[:])
            ones2 = cpool.tile([1, 2], f32)
            nc.vector.memset(ones2[:], 1.0)
            idtF = cpool.tile([P, P], f32)
            make_identity(nc, idtF[:])
            idtH = cpool.tile([P, P], fp16)
            nc.vector.tensor_copy(out=idtH[:], in_=idtF[:])

            # EW2'[v] = edge[v] @ W2.T + b, stacked [2, 128] fp16
            pe = pB.tile([2, D], f32, tag="pb")
            nc.tensor.matmul(out=pe[:], lhsT=edgt_sb[:],
                             rhs=wt_sb[:, D:2 * D], start=True, stop=False)
            nc.tensor.matmul(out=pe[:], lhsT=ones2[:], rhs=b_sb[:],
                             start=False, stop=True)
            ewh = cpool.tile([2, D], fp16)
            nc.vector.tensor_copy(out=ewh[:], in_=pe[:])

            TAc, TBc = [], []
            for c in range(NC_):
                lhs = tblT_sb[:, c * P:(c + 1) * P]
                pa = pB.tile([P, D], f32, tag="pb")
                nc.tensor.matmul(out=pa[:], lhsT=lhs, rhs=wt_sb[:, 0:D],
                                 start=True, stop=True)
                ta = cpool.tile([P, D], fp16, tag=f"ta{c}")
                nc.vector.tensor_copy(out=ta[:], in_=pa[:])
                TAc.append(ta)
                pb_ = pB.tile([P, D], f32, tag="pb")
                nc.tensor.matmul(out=pb_[:], lhsT=lhs, rhs=wt_sb[:, 2 * D:],
                                 start=True, stop=True)
                tb = cpool.tile([P, D], fp16, tag=f"tb{c}")
                nc.vector.tensor_copy(out=tb[:], in_=pb_[:])
                TBc.append(tb)

            # ---------------- flat indices, natural [p, u] layout ----------
            sti = cpool.tile([P, U], i32)
            sii = cpool.tile([P, U], i32)
            rli = cpool.tile([P, U], i32)
            dti = cpool.tile([P, U], i32)
            dii = cpool.tile([P, U], i32)
            for tl, h in ((sti, s_t), (sii, s_i), (rli, r_l),
                          (dti, d_t), (dii, d_i)):
                nc.sync.dma_start(out=tl[:],
                                  in_=h[:].rearrange("(p u) -> p u", p=P))
            nc.vector.tensor_scalar(out=sti[:], in0=sti[:], scalar1=8,
                                    scalar2=None, op0=AL.logical_shift_left)
            nc.vector.tensor_tensor(out=sti[:], in0=sti[:], in1=sii[:],
                                    op=AL.add)
            nc.vector.tensor_scalar(out=dti[:], in0=dti[:], scalar1=8,
                                    scalar2=None, op0=AL.logical_shift_left)
            nc.vector.tensor_tensor(out=dti[:], in0=dti[:], in1=dii[:],
                                    op=AL.add)
            fsh = cpool.tile([P, U], fp16)
            fdh = cpool.tile([P, U], fp16)
            rlh = cpool.tile([P, U], fp16)
            nc.vector.tensor_copy(out=fsh[:], in_=sti[:])
            nc.vector.tensor_copy(out=fdh[:], in_=dti[:])
            nc.vector.tensor_copy(out=rlh[:], in_=rli[:])

            # ---------------- main loop ----------------
            for sg in range(SG):
                u0 = sg * 16
                # broadcast fs/fd across partitions -> [128, 2048] fp16
                FSb = xpool.tile([P, 16 * P], fp16, tag="fsb")
                FDb = xpool.tile([P, 16 * P], fp16, tag="fdb")
                for src, dst in ((fsh, FSb), (fdh, FDb)):
                    for q in range(UQ):
                        pb_ = pB.tile([P, 4 * P], fp16, tag="pb")
                        for k in range(4):
                            u = u0 + q * 4 + k
                            nc.tensor.transpose(
                                out=pb_[:, k * P:(k + 1) * P],
                                in_=src[:, u:u + 1].to_broadcast([P, P]),
                                identity=idtH[:])
                        nc.scalar.copy(out=dst[:, q * 4 * P:(q + 1) * 4 * P],
                                       in_=pb_[:])
                # rel one-hot rows [2, 512] per q
                ohrs = []
                for q in range(UQ):
                    pb_ = pB.tile([P, 4 * P], fp16, tag="pb")
                    for k in range(4):
                        u = u0 + q * 4 + k
                        nc.tensor.transpose(
                            out=pb_[:, k * P:(k + 1) * P],
                            in_=rlh[:, u:u + 1].to_broadcast([P, P]),
                            identity=idtH[:])
                    ohr = ohrp.tile([2, 4 * P], fp16, tag="ohr")
                    nc.vector.tensor_scalar(out=ohr[:], in0=pb_[0:2, :],
                                            scalar1=ioc_sb[0:2, 0:1],
                                            scalar2=None, op0=AL.is_equal)
                    ohrs.append(ohr)

                pos = [pO.tile([P, 4 * P], f32, tag="po", name=f"po{q}")
                       for q in range(UQ)]
                for c in range(NC_):
                    oh = ohp.tile([P, 16 * P], fp16, tag="oh")
                    nc.vector.tensor_scalar(out=oh[:], in0=FSb[:],
                                            scalar1=ioc_sb[:, c:c + 1],
                                            scalar2=None, op0=AL.is_equal)
                    for q in range(UQ):
                        nc.tensor.matmul(out=pos[q][:], lhsT=TAc[c][:],
                                         rhs=oh[:, q * 4 * P:(q + 1) * 4 * P],
                                         start=(c == 0), stop=False)
                for c in range(NC_):
                    oh = ohp.tile([P, 16 * P], fp16, tag="oh")
                    nc.vector.tensor_scalar(out=oh[:], in0=FDb[:],
                                            scalar1=ioc_sb[:, c:c + 1],
                                            scalar2=None, op0=AL.is_equal)
                    for q in range(UQ):
                        nc.tensor.matmul(out=pos[q][:], lhsT=TBc[c][:],
                                         rhs=oh[:, q * 4 * P:(q + 1) * 4 * P],
                                         start=False, stop=False)
                for q in range(UQ):
                    nc.tensor.matmul(out=pos[q][:], lhsT=ewh[:],
                                     rhs=ohrs[q][:], start=False, stop=True)

                # transpose [d, t] -> [t, d] and store
                for q in range(UQ):
                    sbO = osp.tile([P, 4 * P], fp16, tag="sbo")
                    nc.scalar.copy(out=sbO[:], in_=pos[q][:])
                    pt = pT.tile([P, 4 * P], fp16, tag="pt")
                    for k in range(4):
                        nc.tensor.transpose(out=pt[:, k * P:(k + 1) * P],
                                            in_=sbO[:, k * P:(k + 1) * P],
                                            identity=idtH[:])
                    stg = osp.tile([P, 4, P], f32, tag="stg")
                    nc.vector.tensor_copy(
                        out=stg[:].rearrange("m k d -> m (k d)"), in_=pt[:])
                    uq = u0 + q * 4
                    nc.sync.dma_start(out=ov[:, uq:uq + 4, :], in_=stg[:])

    nc.compile()
    return nc


def _make_in_maps(inputs):
    gate = np.asarray(inputs["gate_emb"], np.float32)
    edge = np.asarray(inputs["edge_emb"], np.float32)
    W = np.asarray(inputs["W"], np.float32)
    b = np.asarray(inputs["b"], np.float32)

    tbl = np.concatenate([
        _sinusoid(NI, D),
        _sinusoid(NI, D),
        gate[:NI],
        np.zeros((ROWS - 3 * NI, D), np.float32),
    ], axis=0)  # [1024, 128]

    ioc = (np.arange(128, dtype=np.float32)[:, None]
           + 128.0 * np.arange(8, dtype=np.float32)[None, :])
    common = {
        "ioc": ioc,
        "tblT": np.ascontiguousarray(tbl.T),
        "wt": np.ascontiguousarray(W.T),
        "edget": np.ascontiguousarray(edge.T),
        "bvec": np.ascontiguousarray(b.reshape(1, D)),
    }
    idx_names = ["src_idx", "src_type", "rel", "dst_idx", "dst_type"]
    idx = {k: np.ascontiguousarray(np.asarray(inputs[k]).astype(np.int32))
           for k in idx_names}

    in_maps = []
    for c in range(NCORES):
        m = dict(common)
        for k in idx_names:
            m[k] = np.ascontiguousarray(idx[k][c * NSHARD:(c + 1) * NSHARD])
        in_maps.append(m)
    return in_maps


def kernel(**inputs):
    from concourse.bass_utils import run_bass_kernel_spmd

    if "nc" not in _CACHE:
        _CACHE["nc"] = (_build_nc_onehot() if IMPL == "onehot"
                        else _build_nc())
    nc = _CACHE["nc"]

    in_maps = _make_in_maps(inputs)
    res = run_bass_kernel_spmd(nc, in_maps, core_ids=list(range(NCORES)))
    return np.concatenate([res.results[c]["out"] for c in range(NCORES)],
                          axis=0)
